# revision 61
# baseline (speedup 1.0000x reference)
"""FFTEmbedding kernel for Trainium2 (8 NeuronCores, SPMD data-parallel over B).

Math: per (b, t): out = rfft(x_pad[b, t:t+W]) projected by weight + bias.
Linear in x, so it collapses to a causal conv with M2[w, e] (256, 512):
    out[b, t, e] = sum_w x_pad[b, t+w] * M2[w, e] + bias[e]

Design (per core: 2 batch rows, weights replicated; PE floor = 256 MMs of
128x128x512 fp16 at the measured warm rate ~216 ns = 55.3us):
  * WEIGHT-STATIONARY orientation: out tile = [e_blk 128, t 512] in PSUM.
    lhsT = M2 block [w 128, e 128] (8 distinct tiles), rhs = Hankel slice
    [w 128, t 512].  Hank[p, c] = x_pad[b, p + c] (mega-Hankel SBUF image).
  * OUTPUT IS float8_e3m4 at OUT_SCALE=1/8 (host decodes x8 to fp32).
    The old fp16 output (16.8 MB/core) made the DMA bus (~360 GB/s, all
    16 queues) a co-bottleneck with the PE; e3m4 halves it to 8.4 MB and
    the kernel is cleanly PE-bound.  e3m4 quantization costs ~1.32% rel
    err (gate 2e-2); e3m4 max normal is 15.5 and |out| <= ~70, so the
    1/8 scale leaves ~1.8x headroom (values past 15.5 become inf!).
  * Evacuation fuses scale+bias+e3m4-cast in ONE op per region:
    DVE tensor_scalar(dst, psum, 0.125, bias_vec, mult, add) or ACT
    activation(dst, psum, Identity, bias=bias_vec, scale=0.125), with
    bias pre-scaled by OUT_SCALE on host.  Bias rides the w2 DMA as 4
    extra f16 columns (a separate [128,4] f32 transfer arrived late and
    stalled the first evacuation); a one-time DVE copy upcasts to f32
    (engines require f32 scalar-AP operands for mult).
  * fp8 COMPUTE was evaluated and is a dead end: HW DoubleRow e4m3 MM
    measured 216ns for K=256/N=512 (2x fp16 FLOPs, not the cost model's
    4x), and accuracy needs >= 3 hi/lo GEMM terms (1-term 3.8%, 2-term
    2.7% vs 2e-2 gate) -> 3x27.6us > fp16's 55.3us.  DR numerics/layout
    do work on HW (see microbench.py) if the gate ever loosens.
  * Loop: row-outer, sp-outer (8 seg-pairs of 1024 t), eb-inner.  PSUM
    pool = 4 x [128,1024] f32 tiles (8 banks), recycled per seg-pair.
    PSUM WAR is tile-granular: an in-flight evac read blocks the next
    seg's MM write on the SAME tile (+1.3us PE stall) - the last two
    seg-pairs use per-seg tiles for this reason.
  * Input ladder: row0 chunks [1024, 2048, 2048, 3072] load directly (128
    shifted reads of x -> Hankel image); the load window is DMA-queue
    bound (~150-200 GB/s effective for 2-4KB descriptors incl. fetch
    overhead), so the stream-gating set (c0+w2 ~550KB, split across the
    sync+scalar rings in consumption order) lands ~12us and later chunks
    ride behind with slack.  Row1 (needed ~+27us) is dep-gated into the
    loop: k=32 stage1 on SWDGE + 3 shifted SBUF->SBUF copies on sync.
  * PE warm-up: HAM clock gate needs ~3.4us of sustained PE activity;
    48 junk N=128 matmuls bridge boot(5.8us)+gate-load so the real
    stream starts warm at ~12.1us (swept 43/45/48/53: 48 best, and the
    tight junk->data join also collapsed run-to-run variance).
  * Tail: the final seg-pair's evacs are spread DVE/ACT per-seg so only
    the last [512]'s evac (column-split across both engines) trails the
    last MM; its out-wave is partition-split across sync+scalar (64
    descs each - DIRECT2D issue costs ~0.6us/128-desc ON THE ISSUING
    ENGINE, so waves to evac engines stall evacs).  Non-tail waves ride
    the otherwise-idle SWDGE(gpsimd) queue.  Fixed end cost: last DMA +
    0.9us sem prop + ~2us barrier drain.
Measured (this env): ~74.3us median core-0, all-core mean 74.1 / max
74.9, rel err 1.33e-2 (prior session's fp16-out baseline: 82.5us here,
95.8us on the grading harness).  Real-MM stream is 55.7us wall vs the
55.3us PE floor - remaining slack is the 12.1us head (5.8 boot +
DMA-rate-bound gate load) and ~6.3us tail (evac chain + 0.9 sem + ~2us
barrier drain), both probed to near their floors.  Tail evac engine
assignment matters at the ~0.4us level: ACT must not carry eb3-s0's
evac right before its final half (eb1-s1->ACT, eb3-s0->DVE won the
sweep).  Measure 5+ runs before believing a delta.
"""

import os
import sys

import numpy as np

_TRN_REPO = "/opt/trn_rl_repo"
if _TRN_REPO not in sys.path:
    sys.path.insert(0, _TRN_REPO)

B, T, W_SIZE, EMB = 16, 8192, 256, 512
N_CORES = 8
B_PER = B // N_CORES          # 2 batch rows per core
PAD = W_SIZE - 1              # 255 leading zeros
XP_LEN = T + PAD + 1          # 8448 (one trailing pad elem)

# t-space chunks of the Hankel image per row; chunk j covers t in
# [OFF[b][j], OFF[b][j+1]).  Boundaries must be multiples of 512.
# k per chunk: 128 = direct HBM load of all 128 partitions; k<128 = load
# partitions [0:k] from HBM (stage1) then (128/k - 1) SBUF->SBUF copies
# with col shifts (stage2) on HWDGE rings (SWDGE/gpsimd delivers ~10us
# late - only OK for non-latency-critical transfers).
# row0 gates the matmul stream: a 1024-t gate chunk (small = early
# stream start) then progressively wider chunks (bigger descriptors
# amortize DMA fetch overhead); each lands before the stream reaches it
# modulo ~1us arrival jitter.  row1 has ~27us slack: one chunk, k=32
# staged (4x less HBM read).
CHUNKS = {0: [1024, 2048, 2048, 3072], 1: [8192]}
OFF = {0: [0, 1024, 3072, 5120, 8192], 1: [0, 8192]}
KSTAGE = {0: [128, 128, 128, 128], 1: [32]}

N_SEG = T // 512              # 16 segs of 512 t per row
N_SP = N_SEG // 2             # 8 seg-pairs of 1024 t

# Output is stored as float8_e3m4 scaled by OUT_SCALE (see out_h decl).
OUT_SCALE = 0.125

TRACE = os.environ.get("KERNEL_TRACE", "0") == "1"
# Junk MMs (~95ns each) bridge the ~4.9us between first-possible PE
# activity (~7.4us: boot + gpsimd memset + sem) and gate-data arrival
# (~12.3us), keeping the HAM clock gate warm (needs ~3.4us of sustained
# PE activity); 48 ends ~12.0us, just before typical data arrival.
# A <=1.3us PE idle gap does NOT de-warm the clock (measured).
N_WARM = int(os.environ.get("KERNEL_WARM", "48"))
LAST_RESULT = None

_CACHE = {}


def _build_m2(weight: np.ndarray) -> np.ndarray:
    """(EMB, 258) projection -> (W, EMB) causal-conv matrix, in float64."""
    k = np.arange(W_SIZE // 2 + 1, dtype=np.float64)   # 129
    w = np.arange(W_SIZE, dtype=np.float64)            # 256
    ang = 2.0 * np.pi * np.outer(k, w) / W_SIZE        # (129, 256)
    f = np.concatenate([np.cos(ang), -np.sin(ang)], axis=0)  # (258, 256)
    m2 = (weight.astype(np.float64) @ f).T             # (256, EMB)
    return np.ascontiguousarray(m2, dtype=np.float64)


def _build_program():
    from concourse import bacc, mybir, tile
    from concourse.ap import AP

    f32 = mybir.dt.float32
    f16 = mybir.dt.float16
    f8e3 = mybir.dt.float8e3
    add = mybir.AluOpType.add
    mult = mybir.AluOpType.mult
    ident = mybir.ActivationFunctionType.Identity

    nc = bacc.Bacc(target_bir_lowering=False)
    xpad_h = nc.declare_dram_parameter("xpad", [B_PER, XP_LEN], f16, isOutput=False)
    # w2 packed on host: w2[p, eb*256 + h*128 + m] = M2[128h + p, 128eb + m];
    # cols 1024..1027 carry bias4[p, eb] = bias[128eb + p] * OUT_SCALE in f16
    # (rides the weight DMA - a separate [128,4] transfer costs 128 more
    # descriptors and arrived late enough to stall the first evacuation).
    w2_h = nc.declare_dram_parameter("w2", [128, 2 * EMB + 4], f16, isOutput=False)
    # out stored as e3m4 at OUT_SCALE (host multiplies by 1/OUT_SCALE):
    # halves the output HBM traffic (16.8 -> 8.4 MB/core), which was
    # co-bottleneck with the PE. e3m4 max normal is 15.5; |out| <= ~70, so
    # OUT_SCALE=1/8 keeps the max at ~8.7 with ~1.8x headroom. Measured
    # quantization rel err ~1.3% (gate 2e-2).
    out_h = nc.declare_dram_parameter("out", [B_PER, EMB, T], f8e3, isOutput=True)

    with tile.TileContext(nc) as tc:
        with (
            tc.tile_pool(name="hank", bufs=1) as hank_pool,
            tc.tile_pool(name="wpool", bufs=1) as w_pool,
            tc.tile_pool(name="cpool", bufs=1) as c_pool,
            tc.tile_pool(name="sup", bufs=1) as sup_pool,
            tc.tile_pool(name="psum", bufs=4, space="PSUM") as psum_pool,
        ):
            # ---- PE warm-up: junk matmuls with no input dependency ----
            # memset on gpsimd (otherwise idle); DVE stays clear for evacs.
            # Small N=128 MMs (~107ns cold each): the HAM warm threshold is
            # ~3.4us of SUSTAINED PE activity and the window is free-running,
            # so we overshoot it (36 x 107 = 3.85us) - missing it costs ~6us
            # (the real stream restarts the window); overshoot costs ~100ns
            # per extra junk MM past data arrival.
            junk = c_pool.tile([128, 128], f16, tag="junk")
            nc.gpsimd.memset(junk[:, :], 0.0)
            ps_warm = psum_pool.tile([128, 2 * EMB], f32, name="ps_warm", tag="ps")
            for _ in range(N_WARM):
                nc.tensor.matmul(
                    ps_warm[:, 0:128], junk[:, :], junk[:, :],
                    start=True, stop=True,
                )

            # ---- constants / weights (+ bias in cols 1024..1027) ----
            w01 = w_pool.tile([128, 2 * EMB + 4], f16, tag="w01")
            # engines need the bias scalar-AP in f32: tiny one-time upconvert
            # (emitted after the gate DMAs; fires when w2's 2nd half lands)
            bias32 = c_pool.tile([128, 4], f32, tag="bias32")

            def wslice(h, eb):
                lo = eb * 256 + h * 128
                return w01[:, lo : lo + 128]

            def bvec_of(eb):
                return bias32[:, eb : eb + 1]

            # ---- Hankel images (one per batch row) ----
            # tile width: len + 128 (h=1 reach); staged chunks + (128-k)
            # more so stage2 copies read within the tile.
            hank = {b: [None] * len(CHUNKS[b]) for b in range(B_PER)}

            def make_chunk_tiles(b):
                for j, ln in enumerate(CHUNKS[b]):
                    k = KSTAGE[b][j]
                    w = ln + 128 + (128 - k)
                    hank[b][j] = hank_pool.tile(
                        [128, w], f16, tag=f"hk{j}_{b}", name=f"hk{j}_{b}"
                    )

            def stage1(b, j, eng, p_lo=0, p_hi=None):
                t = hank[b][j]
                k = KSTAGE[b][j]
                if p_hi is None:
                    p_hi = k
                base = b * XP_LEN + OFF[b][j] + p_lo
                cols1 = CHUNKS[b][j] + 128 + (128 - k)
                eng.dma_start(
                    t[p_lo:p_hi, :cols1],
                    AP(xpad_h, base, [[1, p_hi - p_lo], [1, cols1]]),
                )

            def stage2(b, j, eng):
                t = hank[b][j]
                k = KSTAGE[b][j]
                cols2 = CHUNKS[b][j] + 128
                for m in range(1, 128 // k):
                    eng.dma_start(
                        t[k * m : k * (m + 1), 0:cols2],
                        t[0:k, k * m : k * m + cols2],
                    )

            make_chunk_tiles(0)
            make_chunk_tiles(1)
            # Ring discipline: two HWDGE rings (sync + scalar), each chunk
            # split in partition halves across both so per-ring FIFO delivers
            # chunks in consumption order.  The load window is DMA-bus-bound
            # (~360 GB/s aggregate), so the stream-gating set is kept small:
            # c0 is only 512 t (ladder chunks 512/512/1024/2048/2048/2048),
            # and w2ab (with the bias riding in its last 4 columns) lands
            # right behind it.  Later chunks all have >5us of deadline slack.
            # Row1 is dep-gated into the loop (SWDGE + sync stage2).
            # symmetric rings: first-MM set {w2a, c0} splits ~275KB/ring and
            # w2b (eb2/3 weights + bias) lands at the same moment
            nc.sync.dma_start(w01[:, 0:512], w2_h[:, 0:512])
            stage1(0, 0, nc.scalar, 64, 128)       # c0 partitions [64:128]
            stage1(0, 0, nc.sync, 0, 64)           # c0 partitions [0:64]
            nc.scalar.dma_start(w01[:, 512:], w2_h[:, 512:])
            stage1(0, 1, nc.sync, 0, 64)           # c1 halves
            stage1(0, 1, nc.scalar, 64, 128)
            stage1(0, 2, nc.sync, 0, 64)           # c2 halves
            stage1(0, 2, nc.scalar, 64, 128)
            stage1(0, 3, nc.sync, 0, 64)           # c3 halves
            stage1(0, 3, nc.scalar, 64, 128)
            nc.vector.tensor_copy(bias32[:, :], w01[:, 2 * EMB : 2 * EMB + 4])

            def rhs(b, t0, h):
                """Hankel slice [w 128, t 512] for seg at t0, K-half h."""
                for j in range(len(CHUNKS[b])):
                    if t0 < OFF[b][j + 1]:
                        c0 = t0 - OFF[b][j] + 128 * h
                        return hank[b][j][:, c0 : c0 + 512]
                raise AssertionError(t0)

            # ---- sup (output staging) tiles, e3m4 at OUT_SCALE ----
            sup = [
                [
                    sup_pool.tile([128, T], f8e3, tag=f"sup{b}_{eb}", name=f"sup{b}_{eb}")
                    for eb in range(4)
                ]
                for b in range(B_PER)
            ]

            # ---- main loop ----
            def out_dma(eng, b, eb, lo, hi):
                eng.dma_start(
                    out_h[b, eb * 128 : (eb + 1) * 128, lo:hi],
                    sup[b][eb][:, lo:hi],
                )

            def mm_pair(b, eb, ps, t0, s):
                """The two K-half matmuls of one 512-t seg into ps slice."""
                pslice = ps[:, s * 512 : (s + 1) * 512]
                nc.tensor.matmul(
                    pslice, wslice(0, eb), rhs(b, t0 + 512 * s, 0),
                    start=True, stop=False,
                )
                nc.tensor.matmul(
                    pslice, wslice(1, eb), rhs(b, t0 + 512 * s, 1),
                    start=False, stop=True,
                )

            # ---- row 0: sp-outer (Hankel chunks stream in) ----
            b = 0
            for sp in range(N_SP):
                t0 = 1024 * sp
                for eb in (0, 1, 2, 3):
                    bvec = bvec_of(eb)
                    ps = psum_pool.tile(
                        [128, 2 * EMB], f32, name=f"ps_{b}_{sp}_{eb}", tag="ps"
                    )
                    for s in range(2):
                        mm_pair(b, eb, ps, t0, s)
                    dst = sup[b][eb][:, t0 : t0 + 1024]
                    if eb < 2:
                        nc.vector.tensor_scalar(
                            dst, ps[:, :], 0.125, bvec, mult, add
                        )
                    else:
                        nc.scalar.activation(
                            dst, ps[:, :], ident, bias=bvec, scale=0.125
                        )
                # out-DMA waves ride the otherwise-idle SWDGE (gpsimd) ring
                # except eb2/3 on sync (one queue alone was measured to
                # backlog; the final barrier waits for every wave).
                if sp in (1, 3):
                    for eb in range(4):
                        eng = nc.gpsimd if eb < 2 else nc.sync
                        out_dma(eng, b, eb, t0 - 1024, t0 + 1024)
                elif sp >= 4:
                    for eb in range(4):
                        eng = nc.gpsimd if eb < 2 else nc.sync
                        out_dma(eng, b, eb, t0, t0 + 1024)
                if sp == 0:
                    # Row1's stage1 must stay OUT of the critical early
                    # HBM window (eagerly-issued bulk was measured to
                    # starve the stream-gating transfers by ~5us).  The
                    # scheduler hoists ready DMAs, so gate it with a real
                    # dependency: this 2-elem copy depends on sp0's first
                    # evacuation, and the DMA's write-after-write on its
                    # tile makes it wait.
                    nc.vector.tensor_copy(
                        hank[1][0][0:1, 0:2], sup[0][0][0:1, 0:2]
                    )
                    stage1(1, 0, nc.gpsimd)
                    stage2(1, 0, nc.sync)

            # ---- row 1: sp-outer, with a tail-engineered final 2 seg-pairs --
            b = 1
            for sp in range(N_SP - 2):
                t0 = 1024 * sp
                for eb in (0, 1, 2, 3):
                    bvec = bvec_of(eb)
                    ps = psum_pool.tile(
                        [128, 2 * EMB], f32, name=f"ps_{b}_{sp}_{eb}", tag="ps"
                    )
                    for s in range(2):
                        mm_pair(b, eb, ps, t0, s)
                    dst = sup[b][eb][:, t0 : t0 + 1024]
                    if eb < 2:
                        nc.vector.tensor_scalar(
                            dst, ps[:, :], 0.125, bvec, mult, add
                        )
                    else:
                        nc.scalar.activation(
                            dst, ps[:, :], ident, bias=bvec, scale=0.125
                        )
                if sp in (1, 3, 5):
                    for eb in range(4):
                        eng = nc.gpsimd if eb < 2 else nc.sync
                        out_dma(eng, b, eb, t0 - 1024, t0 + 1024)
            # ---- last two seg-pairs: per-seg psum tiles + per-seg evacs.
            # PSUM WAR is tile-granular (an in-flight evac read blocks the
            # next seg's MM write on the same tile, measured +1.3us), so
            # every seg gets its own tile (2-bank tile, low half used).
            # Engine schedule sized so only eb3-s1's evac trails the last
            # MM: DVE 0.66/[512]-evac, ACT 0.69.  Waves: gpsimd carries all
            # [2048] column blocks; the final 1024 columns are partition-
            # split across sync+scalar (64 descs each, parallel ~0.3 issue).
            def seg_unit(sp, eb, s, evac_eng):
                t0 = 1024 * sp
                ts0 = t0 + 512 * s
                ps = psum_pool.tile(
                    [128, 2 * EMB], f32, name=f"ps_{b}_{sp}_{eb}_{s}", tag="ps"
                )
                mm_pair(b, eb, ps, t0, s)
                pslice = ps[:, s * 512 : (s + 1) * 512]
                dst = sup[b][eb][:, ts0 : ts0 + 512]
                if evac_eng == "dve":
                    nc.vector.tensor_scalar(dst, pslice, 0.125, bvec_of(eb),
                                            mult, add)
                else:
                    nc.scalar.activation(dst, pslice, ident, bias=bvec_of(eb),
                                         scale=0.125)

            sp6 = N_SP - 2
            sp7 = N_SP - 1
            for eb in (0, 1, 2, 3):
                seg_unit(sp6, eb, 0, "dve" if eb % 2 == 0 else "act")
                seg_unit(sp6, eb, 1, "act" if eb % 2 == 0 else "dve")
                # sp6 waves issue immediately (during sp7's MM stream) so
                # the tail's gpsimd queue only carries small sp7 waves
                out_dma(nc.gpsimd, b, eb, 1024 * sp6, 1024 * sp6 + 1024)
            # final seg-pair; MM order eb2, eb0, eb1, eb3-s0, eb3-s1;
            # per-eb [1024] waves issue as soon as that eb's evacs land
            seg_unit(sp7, 2, 0, "dve")
            seg_unit(sp7, 2, 1, "act")
            out_dma(nc.gpsimd, b, 2, 1024 * sp7, 1024 * sp7 + 1024)
            seg_unit(sp7, 0, 0, "dve")
            seg_unit(sp7, 0, 1, "dve")
            out_dma(nc.gpsimd, b, 0, 1024 * sp7, 1024 * sp7 + 1024)
            seg_unit(sp7, 1, 0, "act")
            seg_unit(sp7, 1, 1, "act")
            out_dma(nc.gpsimd, b, 1, 1024 * sp7, 1024 * sp7 + 1024)
            seg_unit(sp7, 3, 0, "dve")
            # final seg: column-split evac across ACT+DVE, then one
            # [64-partition, 1024-col] wave per HWDGE ring covering eb3's
            # last two segs
            t0 = 1024 * sp7
            ts0 = t0 + 512
            ps = psum_pool.tile(
                [128, 2 * EMB], f32, name="ps_final", tag="ps"
            )
            mm_pair(b, 3, ps, t0, 1)
            pslice = ps[:, 512:1024]
            dst = sup[b][3][:, ts0 : ts0 + 512]
            nc.scalar.activation(
                dst[:, 0:256], pslice[:, 0:256],
                ident, bias=bvec_of(3), scale=0.125,
            )
            nc.vector.tensor_scalar(
                dst[:, 256:512], pslice[:, 256:512],
                0.125, bvec_of(3), mult, add,
            )
            nc.sync.dma_start(
                out_h[b, 3 * 128 : 3 * 128 + 64, t0 : t0 + 1024],
                sup[b][3][0:64, t0 : t0 + 1024],
            )
            nc.scalar.dma_start(
                out_h[b, 3 * 128 + 64 : 4 * 128, t0 : t0 + 1024],
                sup[b][3][64:128, t0 : t0 + 1024],
            )

    nc.finalize()
    return nc


def _get_program():
    if "prog" not in _CACHE:
        _CACHE["prog"] = _build_program()
    return _CACHE["prog"]


def kernel(x: np.ndarray, weight: np.ndarray, bias: np.ndarray) -> np.ndarray:
    global LAST_RESULT
    from concourse.bass_utils import run_bass_kernel_spmd

    x = np.asarray(x, dtype=np.float32)
    weight = np.asarray(weight, dtype=np.float32)
    bias = np.asarray(bias, dtype=np.float32)

    m2 = _build_m2(weight)
    xpad = np.zeros((B, XP_LEN), dtype=np.float32)
    xpad[:, PAD : PAD + T] = x
    # w2[p, eb*256 + h*128 + m] = M2[128h + p, 128eb + m]; cols 1024..1027
    # carry bias4[p, eb] = bias[128eb + p] * OUT_SCALE (evac computes
    # psum*OUT_SCALE + bias4).
    w2_core = m2.reshape(2, 128, 4, 128).transpose(1, 2, 0, 3).reshape(128, 2 * EMB)
    bias4 = np.ascontiguousarray(bias.reshape(4, 128).T).astype(np.float64) * OUT_SCALE
    w2_in = np.ascontiguousarray(
        np.concatenate([w2_core, bias4], axis=1)
    ).astype(np.float16)
    xpad16 = xpad.astype(np.float16)

    nc = _get_program()
    in_maps = [
        {
            "xpad": np.ascontiguousarray(xpad16[c * B_PER : (c + 1) * B_PER]),
            "w2": w2_in,
        }
        for c in range(N_CORES)
    ]
    res = run_bass_kernel_spmd(nc, in_maps, list(range(N_CORES)), trace=TRACE)
    LAST_RESULT = res
    out_bet = np.concatenate(
        [np.asarray(res.results[c]["out"]) for c in range(N_CORES)], axis=0
    )  # (B, EMB, T) e3m4 at OUT_SCALE
    out = out_bet.astype(np.float32).transpose(0, 2, 1) * (1.0 / OUT_SCALE)
    return np.ascontiguousarray(out)

